# revision 31
# baseline (speedup 1.0000x reference)
"""Trainium2 Bass kernel for bilinear attention.

Reference computation (per batch b, group g):
    q   = po_z[b,g] @ M                       [N, D]
    a   = q @ pr_z[b,g].T                     [N, N]
    A   = softmax(a, axis=-1)                 [N, N]
    out = A @ pr_z[b,g]                       [N, D]
Returns (out, A).

Sharding: pure data parallel over the 16 (b, g) pairs -> 2 pairs per core on
8 NeuronCores; M replicated.

Per-core schedule (per pair):
  - load pr natural tiles (kept as rhs of the final matmul)
  - PE-transpose pr -> prT [D, N] (scores rhs)
  - per 512-row group of queries: load po, PE-transpose -> poT slice,
    compute qT slice = M.T @ poT  (scores lhsT)
  - per 128-row chunk i: scores a = qT.T @ prT into PSUM, row max (DVE),
    exp(a - max) with accumulated row sum (ACT), reciprocal, A = E / sum
    (DVE) -> DMA out; PE-transpose E, out = (E.T).T @ pr scaled by 1/sum
    in the PSUM->SBUF copy (ACT) -> DMA out.
  The E-transpose + out matmuls are software-pipelined one chunk behind the
  scores so the PE never waits for the softmax.

Precision: the softmax scores have std ~512, so exp(a - max) on near-tie
rows is visibly perturbed by any low-precision score matmul (bf16/TF32
fail). The scores and q = po @ M matmuls therefore run as 3 full-rate fp16
matmuls over hi/lo splits (x = hi + lo, both fp16; residual ~2^-23 per
product -- the same error class as fp32, but 3 PE cycles/row instead of
fp32's 4). The softmax itself (max, exp, sum, reciprocal, normalize) is
fp32 and matches jax bitwise. Only the E-transpose + out-matmul operands
are plain fp16: the measured absmax error of `out` is dominated by fp32
rounding of the scores, not by 11-bit rounding of E/pr.
"""

import numpy as np

import concourse.bass as bass
import concourse.mybir as mybir
import concourse.tile as tile
from concourse import bacc
from concourse.bass_utils import run_bass_kernel_spmd
from concourse.masks import make_identity

F32 = mybir.dt.float32
F32R = mybir.dt.float32r
AX = mybir.AxisListType.X
EXP = mybir.ActivationFunctionType.Exp
COPY = mybir.ActivationFunctionType.Copy

F16 = mybir.dt.float16
SUB = mybir.AluOpType.subtract

# dtype for the E-transpose + out = (E @ pr) / s matmul operands. fp16
# (11-bit mantissa, full-rate PE, 1cyc/row transposes) measures the same
# error as fp32 here: the out absmax error is dominated by fp32 rounding of
# the softmax scores, not by 11-bit rounding of E/pr. fp32 is the fallback.
OUT_MM_DTYPE = F16

# When True, the scores (and q = po @ M) matmuls run as 3 full-rate fp16
# matmuls over hi/lo splits (x = hi + lo, both fp16) instead of one 4x-cost
# fp32 matmul. Residual per product ~2^-23 -- same error class as fp32.
SCORES_SPLIT = True

B, G, N, D = 2, 8, 2048, 512
NCORES = 8
PAIRS = (B * G) // NCORES  # (b,g) pairs per core
P = 128  # partitions
NI = N // P  # 16 row chunks per pair
ND = D // P  # 4


def _emit_et_out(nc, pools, ident, prn, e_sb, recip, o_ext, p, ic):
    """Transpose E chunk and compute out[ic] = (E @ pr) / rowsum."""
    ps_at, ps_out, at_pool, small_pool = (
        pools["ps_at"],
        pools["ps_out"],
        pools["at"],
        pools["small"],
    )
    if OUT_MM_DTYPE == F32:
        e_t = e_sb
    else:
        e_t = pools["e"].tile([P, N], OUT_MM_DTYPE, tag="e16")
        nc.vector.tensor_copy(e_t[:], e_sb[:])
    at_sb = at_pool.tile([P, NI, P], OUT_MM_DTYPE, tag="atsb")
    for q4 in range(NI // 4):
        pt = ps_at.tile([P, 512], OUT_MM_DTYPE, tag="at")
        for jj in range(4):
            jb = 4 * q4 + jj
            nc.tensor.matmul(
                pt[:, P * jj : P * (jj + 1)],
                e_t[:, P * jb : P * (jb + 1)],
                ident[OUT_MM_DTYPE][:],
                is_transpose=True,
                start=(jj == 0),
                stop=(jj == 3),
            )
        nc.vector.tensor_copy(at_sb[:, 4 * q4 : 4 * q4 + 4, :], pt[:])
    op = ps_out.tile([P, D], F32, tag="pout")
    for jb in range(NI):
        nc.tensor.matmul(
            op[:], at_sb[:, jb, :], prn[jb][:], start=(jb == 0), stop=(jb == NI - 1)
        )
    out_sb = small_pool.tile([P, D], F32, tag="outsb")
    nc.scalar.activation(out_sb[:], op[:], COPY, bias=0.0, scale=recip[:])
    nc.gpsimd.dma_start(o_ext[p, P * ic : P * (ic + 1), :], out_sb[:])


def _emit_pair(nc, pools, ident, mt, po_ext, pr_ext, a_ext, o_ext, p):
    ps_a, ps_at, ps_out = pools["ps_a"], pools["ps_at"], pools["ps_out"]

    # prT bands [128(d), N] (scores rhs) + prn tiles [128, D] in OUT_MM_DTYPE
    # (out-matmul rhs), both fed from fp32 staging tiles. With SCORES_SPLIT
    # the bands are stored as fp16 hi/lo pairs instead of one fp32 band.
    prn = []
    prT = []
    for _dc in range(ND):
        if SCORES_SPLIT:
            bhi = pools["big"].tile([P, N], F16, tag="phi")
            blo = pools["big"].tile([P, N], F16, tag="plo")
            prT.append((bhi, blo))
        else:
            band = pools["big"].tile([P, N], F32, tag="prT")
            prT.append(band)
    for q in range(4):
        prs = []
        for jj in range(4):
            t = pools["pon"].tile([P, D], F32, tag="prs")
            r0 = P * (4 * q + jj)
            nc.sync.dma_start(t[:], pr_ext[p, r0 : r0 + P, :])
            prs.append(t)
        for jj in range(4):
            t16 = pools["prn"].tile([P, D], OUT_MM_DTYPE, tag="prn")
            if OUT_MM_DTYPE == F32:
                nc.scalar.copy(t16[:], prs[jj][:])
            else:
                nc.vector.tensor_copy(t16[:], prs[jj][:])
            prn.append(t16)
        for dc in range(ND):
            pt = ps_at.tile([P, 512], F32, tag="at")
            for jj in range(4):
                nc.tensor.matmul(
                    pt[:, P * jj : P * (jj + 1)],
                    prs[jj][:, P * dc : P * (dc + 1)],
                    ident[F32][:],
                    is_transpose=True,
                    start=(jj == 0),
                    stop=(jj == 3),
                )
            sl = slice(512 * q, 512 * (q + 1))
            if SCORES_SPLIT:
                bhi, blo = prT[dc]
                nc.vector.tensor_copy(bhi[:, sl], pt[:])
                nc.vector.tensor_tensor(blo[:, sl], pt[:], bhi[:, sl], SUB)
            else:
                nc.vector.tensor_copy(prT[dc][:, sl], pt[:])

    pending = []
    for iq in range(4):  # groups of 4 row-chunks (512 query rows)
        pon = []
        for ii in range(4):
            t = pools["pon"].tile([P, D], F32, tag="pon")
            r0 = P * (4 * iq + ii)
            nc.sync.dma_start(t[:], po_ext[p, r0 : r0 + P, :])
            pon.append(t)
        # poT slice [128(d), 512(i)] per d-chunk for this row group
        poTs = []
        for dc in range(ND):
            pt = ps_at.tile([P, 512], F32, tag="at")
            for ii in range(4):
                nc.tensor.matmul(
                    pt[:, P * ii : P * (ii + 1)],
                    pon[ii][:, P * dc : P * (dc + 1)],
                    ident[F32][:],
                    is_transpose=True,
                    start=(ii == 0),
                    stop=(ii == 3),
                )
            if SCORES_SPLIT:
                shi = pools["q"].tile([P, 512], F16, tag="pohi")
                slo = pools["q"].tile([P, 512], F16, tag="polo")
                nc.vector.tensor_copy(shi[:], pt[:])
                nc.vector.tensor_tensor(slo[:], pt[:], shi[:], SUB)
                poTs.append((shi, slo))
            else:
                s = pools["q"].tile([P, 512], F32, tag="poTs")
                nc.vector.tensor_copy(s[:], pt[:])
                poTs.append(s)
        # qT slice [128(e), 512(i)] per e-chunk: qT = M.T @ poT
        qTs = []
        for ec in range(ND):
            qp = ps_out.tile([P, 512], F32, tag="pout")
            if SCORES_SPLIT:
                mhi, mlo = mt
                idx = 0
                for lh, rh in ((0, 0), (0, 1), (1, 0)):
                    for dc in range(ND):
                        lhs = (mhi, mlo)[lh][:, dc, P * ec : P * (ec + 1)]
                        rhs = poTs[dc][rh][:]
                        nc.tensor.matmul(
                            qp[:], lhs, rhs, start=(idx == 0), stop=(idx == 11)
                        )
                        idx += 1
                qhi = pools["q"].tile([P, 512], F16, tag="qhi")
                qlo = pools["q"].tile([P, 512], F16, tag="qlo")
                nc.vector.tensor_copy(qhi[:], qp[:])
                nc.vector.tensor_tensor(qlo[:], qp[:], qhi[:], SUB)
                qTs.append((qhi, qlo))
            else:
                for dc in range(ND):
                    nc.tensor.matmul(
                        qp[:],
                        mt[:, dc, P * ec : P * (ec + 1)],
                        poTs[dc][:],
                        start=(dc == 0),
                        stop=(dc == ND - 1),
                    )
                s = pools["q"].tile([P, 512], F32, tag="qTs")
                nc.vector.tensor_copy(s[:], qp[:])
                qTs.append(s)

        for ii in range(4):
            ic = 4 * iq + ii
            # scores a[ic] = q[ic] @ pr.T  -> PSUM [128, N]
            ap = ps_a.tile([P, N], F32, tag="pa")
            # lhsT-major order: each stationary operand is reused across the
            # 4 jq matmuls (4x fewer distinct LDWEIGHTS on the PE)
            # per-bank row max (mx4[:, jq]) chases each PSUM bank as its
            # accumulation group closes, so the DVE max mostly overlaps the
            # remaining score matmuls instead of serializing after them
            mx4 = pools["small"].tile([P, 4], F32, tag="mx4")
            if SCORES_SPLIT:
                for jq in range(4):
                    idx = 0
                    for lh, rh in ((0, 0), (0, 1), (1, 0)):
                        for ec in range(ND):
                            nc.tensor.matmul(
                                ap[:, 512 * jq : 512 * (jq + 1)],
                                qTs[ec][lh][:, P * ii : P * (ii + 1)],
                                prT[ec][rh][:, 512 * jq : 512 * (jq + 1)],
                                start=(idx == 0),
                                stop=(idx == 11),
                            )
                            idx += 1
                    nc.vector.reduce_max(
                        mx4[:, jq : jq + 1],
                        ap[:, 512 * jq : 512 * (jq + 1)],
                        axis=AX,
                    )
            else:
                for idx, ec in enumerate(range(ND)):
                    lhs = qTs[ec][:, P * ii : P * (ii + 1)]
                    for jq in range(4):
                        nc.tensor.matmul(
                            ap[:, 512 * jq : 512 * (jq + 1)],
                            lhs,
                            prT[ec][:, 512 * jq : 512 * (jq + 1)],
                            start=(idx == 0),
                            stop=(idx == ND - 1),
                        )
                for jq in range(4):
                    nc.vector.reduce_max(
                        mx4[:, jq : jq + 1],
                        ap[:, 512 * jq : 512 * (jq + 1)],
                        axis=AX,
                    )
            negmax = pools["small"].tile([P, 1], F32, tag="negmax")
            nc.vector.reduce_max(negmax[:], mx4[:], axis=AX, negate=True)
            e_sb = pools["e"].tile([P, N], F32, tag="esb")
            esum = pools["small"].tile([P, 1], F32, tag="esum")
            nc.scalar.activation(
                e_sb[:], ap[:], EXP, bias=negmax[:], scale=1.0, accum_out=esum[:]
            )
            recip = pools["small"].tile([P, 1], F32, tag="recip")
            nc.vector.reciprocal(recip[:], esum[:])
            a_sb = pools["a"].tile([P, N], F32, tag="asb")
            nc.vector.tensor_scalar_mul(a_sb[:], e_sb[:], recip[:])
            nc.gpsimd.dma_start(a_ext[p, P * ic : P * (ic + 1), :], a_sb[:])

            pending.append((e_sb, recip, ic))
            if len(pending) > 2:
                pe, pr_, pic = pending.pop(0)
                _emit_et_out(nc, pools, ident, prn, pe, pr_, o_ext, p, pic)

    for pe, pr_, pic in pending:
        _emit_et_out(nc, pools, ident, prn, pe, pr_, o_ext, p, pic)


def build_program(repeat=None):
    nc = bacc.Bacc("TRN2", target_bir_lowering=False, debug=False, num_devices=NCORES)
    po_ext = nc.dram_tensor("po", [PAIRS, N, D], F32, kind="ExternalInput").ap()
    pr_ext = nc.dram_tensor("pr", [PAIRS, N, D], F32, kind="ExternalInput").ap()
    m_ext = nc.dram_tensor("m", [D, D], F32, kind="ExternalInput").ap()
    a_ext = nc.dram_tensor("attn", [PAIRS, N, N], F32, kind="ExternalOutput").ap()
    o_ext = nc.dram_tensor("out", [PAIRS, N, D], F32, kind="ExternalOutput").ap()

    with tile.TileContext(nc) as tc:
        with (
            tc.tile_pool(name="const", bufs=1) as const_pool,
            tc.tile_pool(name="prn", bufs=20) as prn_pool,
            tc.tile_pool(name="big", bufs=ND) as big_pool,
            tc.tile_pool(name="q", bufs=8) as q_pool,
            tc.tile_pool(name="pon", bufs=4) as pon_pool,
            tc.tile_pool(name="e", bufs=3) as e_pool,
            tc.tile_pool(name="a", bufs=2) as a_pool,
            tc.tile_pool(name="at", bufs=2) as at_pool,
            tc.tile_pool(name="small", bufs=3) as small_pool,
            tc.tile_pool(name="ps_a", bufs=1, space="PSUM") as ps_a,
            tc.tile_pool(name="ps_at", bufs=2, space="PSUM") as ps_at,
            tc.tile_pool(name="ps_out", bufs=2, space="PSUM") as ps_out,
        ):
            pools = {
                "prn": prn_pool,
                "big": big_pool,
                "q": q_pool,
                "pon": pon_pool,
                "e": e_pool,
                "a": a_pool,
                "at": at_pool,
                "small": small_pool,
                "ps_a": ps_a,
                "ps_at": ps_at,
                "ps_out": ps_out,
            }
            mt32 = const_pool.tile([P, ND, D], F32, tag="mt")
            nc.sync.dma_start(mt32[:], m_ext.rearrange("(c p) e -> p c e", p=P))
            if SCORES_SPLIT:
                mhi = const_pool.tile([P, ND, D], F16, tag="mhi")
                mlo = const_pool.tile([P, ND, D], F16, tag="mlo")
                nc.vector.tensor_copy(mhi[:], mt32[:])
                nc.vector.tensor_tensor(mlo[:], mt32[:], mhi[:], SUB)
                mt = (mhi, mlo)
            else:
                mt = mt32
            ident32 = const_pool.tile([P, P], F32, tag="ident")
            make_identity(nc, ident32[:])
            ident = {F32: ident32}
            if OUT_MM_DTYPE != F32:
                ident_o = const_pool.tile([P, P], OUT_MM_DTYPE, tag="ident_o")
                nc.vector.tensor_copy(ident_o[:], ident32[:])
                ident[OUT_MM_DTYPE] = ident_o

            def _emit_all():
                for p in range(PAIRS):
                    _emit_pair(nc, pools, ident, mt, po_ext, pr_ext, a_ext, o_ext, p)

            if repeat:
                with tc.For_i(0, repeat, 1):
                    _emit_all()
            else:
                _emit_all()

    nc.compile()
    return nc


_PROGRAM_CACHE = {}


def _get_program():
    if "nc" not in _PROGRAM_CACHE:
        _PROGRAM_CACHE["nc"] = build_program()
    return _PROGRAM_CACHE["nc"]


def run(po_z, pr_z, M, trace=False):
    nc = _get_program()
    po_f = np.ascontiguousarray(po_z.reshape(B * G, N, D), dtype=np.float32)
    pr_f = np.ascontiguousarray(pr_z.reshape(B * G, N, D), dtype=np.float32)
    m_np = np.ascontiguousarray(M, dtype=np.float32)
    in_maps = [
        {
            "po": po_f[PAIRS * c : PAIRS * (c + 1)],
            "pr": pr_f[PAIRS * c : PAIRS * (c + 1)],
            "m": m_np,
        }
        for c in range(NCORES)
    ]
    res = run_bass_kernel_spmd(nc, in_maps, list(range(NCORES)), trace=trace)
    out = np.concatenate([res.results[c]["out"] for c in range(NCORES)]).reshape(
        B, G, N, D
    )
    attn = np.concatenate([res.results[c]["attn"] for c in range(NCORES)]).reshape(
        B, G, N, N
    )
    return (out, attn), res


def kernel(po_z, pr_z, M):
    (out, attn), _ = run(po_z, pr_z, M)
    return (out, attn)
